# revision 9
# baseline (speedup 1.0000x reference)
"""Sparse attention (template/search) Trainium2 Bass kernel.

Problem: B=128, N=320 (T=64 template + S=256 search), C=768, H=12, d=64.
  x = concat(x1[:, :64], x2[:, 64:])
  qkv = x @ qkv_w.T ; per-head attention (template->template, search->all)
  out = attn @ proj_w.T + proj_b
Pure data parallel over batch: 16 batches per core on 8 cores.

Dataflow (per batch, all feature-major / "transposed" on chip, bf16
matmul operands, f32 PSUM accumulation):
  xT [C, N] --wqk--> qkT [2C rows, N]      (12 m-tiles, free dim 320)
  xT --wv--> v token-major per-head 65-wide blocks ([64 v cols | ones])
  scores^T [s, t] per head pair, quadrant-packed (two heads at
  tile_position row offsets 0/64 writing different PSUM banks so they
  stream concurrently); template scores ride the spare columns of the
  third s-tile.  Exp (ACT) -> bf16 tiles; attn @ [v|1] puts the softmax
  numerator in rows 0..63 and the denominator in row 64 of a psA tile.
  Normalization: ACT copies the denominator row to SBUF, DVE
  reciprocal_approx_fast (NOT the 8-cycle/elem iterative divide),
  GpSimd partition-broadcast, DVE multiply -> attn bf16.
  Projection reads attn feature-major; the bias is fused into the DVE
  PSUM->SBUF copy as a per-partition tensor_scalar_add.

Instruction-stream scheduling (the previous version lost ~2x to PE
clock-gate throttling from dependency stalls):
  - attnV for pair p is emitted after the score matmuls of pair p+1, so
    the Exp(p) latency is hidden behind score streaming;
  - proj(b-1) is emitted after qkv(b)+v(b), giving the normalization
    chain of batch b-1's last pairs ~15us of PE work to complete;
  - psA (1-bank [128,512] tiles, bufs=4) is shared by qkv/v/proj
    accumulators and attnV outputs; psB (2-bank [128,1024], bufs=2)
    holds scores. All 8 PSUM banks in use.

PSUM discipline: two matmuls that can execute concurrently on the PE
(disjoint row groups) must never target the same PSUM bank. The only
row-disjoint concurrent writers are the quadrant-packed score matmuls,
which write different banks by construction; every other matmul spans
row 0+ and is row-serialized with its neighbors.
"""

import numpy as np
import ml_dtypes

import concourse.bass as bass
import concourse.bacc as bacc
import concourse.mybir as mybir
from concourse.tile import TileContext
from concourse.bass_utils import run_bass_kernel_spmd

f32 = mybir.dt.float32
bf16 = mybir.dt.bfloat16
i32 = mybir.dt.int32
Exp = mybir.ActivationFunctionType.Exp


B, N, C = 128, 320, 768
H, D = 12, 64
T, S = 64, 256
N_CORES = 8
BPC = B // N_CORES  # batches per core

NCT = C // 128            # 6 c-tiles of 128
NQK = 2 * C // 128        # 12 qk row-tiles
NPAIR = H // 2            # 6 head pairs
S_TILES = [(0, 128), (128, 128), (256, 64)]   # (s0, ssz) key-token tiles
SCALE = D ** -0.5
VW = 65                   # per-head V block width (64 v cols + ones)
NP_BF16 = ml_dtypes.bfloat16


def build_bass(bpc: int = BPC, n_cores: int = N_CORES, reps: int = 1):
    nc = bacc.Bacc("TRN2", target_bir_lowering=False, debug=False,
                   num_devices=n_cores)

    xt_d = nc.declare_dram_parameter("xt", [bpc, C, N], bf16, isOutput=False)
    # host-pretransposed: wqk = qkv_w[:1536].T, wv = qkv_w[1536:].T,
    # wp = proj_w.T  (all [C, m] feature-major, bf16)
    wqk_d = nc.declare_dram_parameter("wqk", [C, 2 * C], bf16, isOutput=False)
    wv_d = nc.declare_dram_parameter("wv", [C, C], bf16, isOutput=False)
    wp_d = nc.declare_dram_parameter("wp", [C, C], bf16, isOutput=False)
    # pbt[p, m] = proj_b[m*128 + p]
    pb_d = nc.declare_dram_parameter("pbt", [128, NCT], f32, isOutput=False)
    r_d = None
    if reps == 0:   # timing harness: runtime iteration count
        r_d = nc.declare_dram_parameter("reps_in", [1, 1], i32, isOutput=False)
    y_d = nc.declare_dram_parameter("y", [bpc, C, N], f32, isOutput=True)

    with TileContext(nc) as tc:
        with (
            tc.tile_pool(name="wpool", bufs=1) as wpool,
            tc.tile_pool(name="xpool", bufs=2) as xpool,
            tc.tile_pool(name="qkpool", bufs=2) as qkpool,
            tc.tile_pool(name="vpool", bufs=2) as vpool,
            tc.tile_pool(name="epool", bufs=6) as epool,
            tc.tile_pool(name="apool", bufs=2) as apool,
            tc.tile_pool(name="rpool", bufs=6) as rpool,
            tc.tile_pool(name="bpool", bufs=6) as bpool,
            tc.tile_pool(name="ypool", bufs=3) as ypool,
            tc.tile_pool(name="psA", bufs=4, space="PSUM") as psA,
            tc.tile_pool(name="psB", bufs=2, space="PSUM") as psB,
        ):
            # ---- persistent weights ----
            wqk_sb = wpool.tile([128, NCT, 2 * C], bf16)   # lhsT for q,k
            nc.sync.dma_start(out=wqk_sb[:],
                              in_=wqk_d.rearrange("(ct p) m -> p ct m", p=128))
            wv_sb = wpool.tile([128, NCT, C], bf16)        # rhs for v
            nc.sync.dma_start(out=wv_sb[:],
                              in_=wv_d.rearrange("(ct p) m -> p ct m", p=128))
            wp_sb = wpool.tile([128, NCT, C], bf16)        # lhsT for proj
            nc.sync.dma_start(out=wp_sb[:],
                              in_=wp_d.rearrange("(ct p) m -> p ct m", p=128))
            pb_sb = wpool.tile([128, NCT], f32)
            nc.sync.dma_start(out=pb_sb[:], in_=pb_d[:])
            rv = None
            if reps == 0:
                r_sb = wpool.tile([1, 1], i32)
                nc.sync.dma_start(out=r_sb[:], in_=r_d[:])
                tmp = nc.alloc_registers("reps_regs")
                nc.regs_load(tmp, r_sb[0:1, 0:1])
                rv = nc.snap(tmp, donate=True, min_val=1, max_val=4096)

            def emit_qkv(xt_sb, qk_sb):
                for m in range(NQK):
                    pacc = psA.tile([128, 512], f32, tag="pacc")
                    for ct in range(NCT):
                        nc.tensor.matmul(
                            pacc[:, 0:N],
                            wqk_sb[:, ct, m * 128:(m + 1) * 128],
                            xt_sb[:, ct, :],
                            start=(ct == 0), stop=(ct == NCT - 1))
                    nc.vector.tensor_copy(qk_sb[:, m, :], pacc[:, 0:N])

            def emit_v(xt_sb, v_sb):
                for tt, tsz in ((0, 128), (1, 128), (2, 64)):
                    for c0, csz in ((0, 512), (512, 256)):
                        pacc = psA.tile([128, 512], f32, tag="pacc")
                        for ct in range(NCT):
                            nc.tensor.matmul(
                                pacc[0:tsz, 0:csz],
                                xt_sb[:, ct, tt * 128:tt * 128 + tsz],
                                wv_sb[:, ct, c0:c0 + csz],
                                start=(ct == 0), stop=(ct == NCT - 1))
                        nh = csz // D
                        h0 = c0 // D
                        dst = v_sb[0:tsz, tt, h0 * VW:(h0 + nh) * VW] \
                            .rearrange("p (h c) -> p h c", c=VW)[:, :, 0:D]
                        src = pacc[0:tsz, 0:csz] \
                            .rearrange("p (h c) -> p h c", c=D)
                        nc.scalar.copy(dst, src)
                ones_dst = v_sb[:, :, :].rearrange(
                    "p t (h c) -> p t h c", c=VW)[:, :, :, D:VW]
                nc.vector.memset(ones_dst, 1.0)

            def emit_scores(qk_sb, p):
                """Score matmuls + Exp for pair p. Returns exp tiles."""
                mq, mk = p, NPAIR + p
                exps = []
                for st, (s0, ssz) in enumerate(S_TILES):
                    last = (st == len(S_TILES) - 1)
                    w = 320 if last else 256
                    sc = psB.tile([128, 1024], f32, tag="sc")
                    for i in (0, 1):
                        pof = 64 * i
                        nc.tensor.matmul(
                            sc[0:ssz, 512 * i:512 * i + 256],
                            qk_sb[pof:pof + 64, mk, s0:s0 + ssz],
                            qk_sb[pof:pof + 64, mq, T:N],
                            start=True, stop=True,
                            tile_position=(pof, 0))
                        if last:
                            nc.tensor.matmul(
                                sc[0:T, 512 * i + 256:512 * i + 320],
                                qk_sb[pof:pof + 64, mk, 0:T],
                                qk_sb[pof:pof + 64, mq, 0:T],
                                start=True, stop=True,
                                tile_position=(pof, 0))
                    ex = epool.tile([128, 2, 320], bf16, tag="ex")
                    gap_in = bass.AP(
                        tensor=sc.tensor, offset=sc.offset,
                        ap=[sc.ap[0], [512, 2], [1, w]])
                    nc.scalar.activation(out=ex[0:ssz, :, 0:w],
                                         in_=gap_in[0:ssz],
                                         func=Exp, scale=SCALE)
                    exps.append(ex)
                return exps

            def emit_av_norm(v_sb, attn_sb, p, exps):
                """attn @ [v | 1] + normalization for pair p.

                Pairs 4-5 write their O tiles into psB (both heads in one
                2-bank tile, heads serialized so bank sharing is safe):
                this way the next batch's qkv psA allocations recycle
                pairs 0-3 (whose muls completed mid-attention) instead of
                stalling ~1.4us on the tail pairs' normalization; the psB
                recycle is absorbed by sc(b+1, p0/p1), emitted a full
                qkv+v+proj (~20us) later."""
                O2 = (psB.tile([128, 1024], f32, tag="sc", name="O2")
                      if p >= 4 else None)
                for i, h in enumerate((2 * p, 2 * p + 1)):
                    if O2 is not None:
                        O = O2[:, 512 * i:512 * i + 512]
                    else:
                        O = psA.tile([128, 512], f32, tag="pacc")
                    for st, (s0, ssz) in enumerate(S_TILES):
                        nc.tensor.matmul(
                            O[0:VW, T:N],
                            v_sb[0:ssz, st, h * VW:(h + 1) * VW],
                            exps[st][0:ssz, i, 0:256],
                            start=(st == 0),
                            stop=(st == len(S_TILES) - 1))
                    nc.tensor.matmul(
                        O[0:VW, 0:T],
                        v_sb[0:T, 0, h * VW:(h + 1) * VW],
                        exps[2][0:T, i, 256:320],
                        start=True, stop=True)

                    # ACT copies the denominator row to SBUF (the custom
                    # DVE reciprocal's bitwise seed misreads PSUM), then
                    # the ~5x-faster approx reciprocal runs on DVE.
                    den = rpool.tile([1, N], f32, tag="den")
                    nc.scalar.copy(den[0:1, :], O[64:65, 0:N])
                    rec = rpool.tile([1, N], f32, tag="rec")
                    nc.vector.reciprocal_approx_fast(out=rec[0:1, :],
                                                     in_=den[0:1, :])
                    rb = bpool.tile([64, N], f32, tag="rb")
                    nc.gpsimd.partition_broadcast(rb[0:64, :], rec[0:1, :])
                    nc.vector.tensor_mul(
                        attn_sb[64 * i:64 * i + 64, p, :],
                        O[0:64, 0:N], rb[0:64, :])

            def emit_proj(attn_sb, b):
                for m in range(NCT):
                    yp = psA.tile([128, 512], f32, tag="pacc")
                    for ct in range(NCT):
                        nc.tensor.matmul(
                            yp[:, 0:N],
                            wp_sb[:, ct, m * 128:(m + 1) * 128],
                            attn_sb[:, ct, :],
                            start=(ct == 0), stop=(ct == NCT - 1))
                    yt_sb = ypool.tile([128, N], f32, tag="yt")
                    nc.vector.tensor_scalar_add(yt_sb[:], yp[:, 0:N],
                                                pb_sb[:, m:m + 1])
                    nc.sync.dma_start(out=y_d[b, m * 128:(m + 1) * 128, :],
                                      in_=yt_sb[:])

            def body(_iv=None):
                prev = None           # (attn_sb, b) awaiting projection
                for b in range(bpc):
                    xt_sb = xpool.tile([128, NCT, N], bf16)
                    nc.sync.dma_start(
                        out=xt_sb[:],
                        in_=xt_d[b].rearrange("(ct p) t -> p ct t", p=128))

                    qk_sb = qkpool.tile([128, NQK, N], bf16)
                    emit_qkv(xt_sb, qk_sb)
                    v_sb = vpool.tile([128, 3, H * VW], bf16)
                    emit_v(xt_sb, v_sb)

                    if prev is not None:
                        emit_proj(*prev)

                    attn_sb = apool.tile([128, NPAIR, N], bf16)
                    pend = None       # (p, exps) with attnV not yet emitted
                    for p in range(NPAIR):
                        exps = emit_scores(qk_sb, p)
                        if pend is not None:
                            emit_av_norm(v_sb, attn_sb, *pend)
                        pend = (p, exps)
                    emit_av_norm(v_sb, attn_sb, *pend)
                    prev = (attn_sb, b)
                emit_proj(*prev)

            if reps == 1:
                body()
            elif reps == 0:
                with tc.For_i(0, rv, 1) as _i:
                    body(_i)
            else:
                with tc.For_i(0, reps, 1) as _i:
                    body(_i)

    nc.compile()
    return nc


_NC_CACHE = {}


def _get_nc(bpc: int = BPC):
    if bpc not in _NC_CACHE:
        _NC_CACHE[bpc] = build_bass(bpc)
    return _NC_CACHE[bpc]


def make_in_maps(x1, x2, qkv_w, proj_w, proj_b, n_cores=N_CORES):
    x1 = np.asarray(x1, dtype=np.float32)
    x2 = np.asarray(x2, dtype=np.float32)
    qkv_w = np.asarray(qkv_w, dtype=np.float32)
    proj_w = np.asarray(proj_w, dtype=np.float32)
    proj_b = np.asarray(proj_b, dtype=np.float32)

    b = x1.shape[0]
    xt = np.empty((b, C, N), dtype=NP_BF16)
    xt[:, :, :T] = x1[:, :T, :].transpose(0, 2, 1).astype(NP_BF16)
    xt[:, :, T:] = x2[:, T:, :].transpose(0, 2, 1).astype(NP_BF16)

    wqk = np.ascontiguousarray(qkv_w[:2 * C].T).astype(NP_BF16)
    wv = np.ascontiguousarray(qkv_w[2 * C:].T).astype(NP_BF16)
    wp = np.ascontiguousarray(proj_w.T).astype(NP_BF16)
    pbt = np.ascontiguousarray(proj_b.reshape(NCT, 128).T)  # [128, NCT] f32

    bpc = b // n_cores
    return [
        {"xt": xt[c * bpc:(c + 1) * bpc], "wqk": wqk, "wv": wv, "wp": wp,
         "pbt": pbt}
        for c in range(n_cores)
    ], bpc


def kernel(x1, x2, qkv_w, proj_w, proj_b):
    in_maps, bpc = make_in_maps(x1, x2, qkv_w, proj_w, proj_b)
    nc = _get_nc(bpc)
    res = run_bass_kernel_spmd(nc, in_maps, list(range(N_CORES)))
    yt = np.concatenate([res.results[c]["y"] for c in range(N_CORES)], axis=0)
    return np.ascontiguousarray(yt.transpose(0, 2, 1))


# revision 16
# speedup vs baseline: 1.0014x; 1.0014x over previous
"""Sparse attention (template/search) Trainium2 Bass kernel.

Problem: B=128, N=320 (T=64 template + S=256 search), C=768, H=12, d=64.
  x = concat(x1[:, :64], x2[:, 64:])
  qkv = x @ qkv_w.T ; per-head attention (template->template, search->all)
  out = attn @ proj_w.T + proj_b
Pure data parallel over batch: 16 batches per core on 8 cores.

Dataflow (per batch, all feature-major / "transposed" on chip, bf16
matmul operands, f32 PSUM accumulation):
  xT [C, N] --wqk--> qkT [2C rows, N]      (12 m-tiles, free dim 320)
  xT --wv--> v token-major per-head 65-wide blocks ([64 v cols | ones])
  scores^T [s, t] per head pair, quadrant-packed (two heads at
  tile_position row offsets 0/64 writing different PSUM banks so they
  stream concurrently); template scores ride the spare columns of the
  third s-tile.  Exp (ACT) -> bf16 tiles; attn @ [v|1] puts the softmax
  numerator in rows 0..63 and the denominator in row 64 of a psA tile.
  Normalization: ACT copies the denominator row to SBUF, DVE
  reciprocal_approx_fast (NOT the 8-cycle/elem iterative divide),
  GpSimd partition-broadcast, DVE multiply -> attn bf16.
  Projection reads attn feature-major; the bias is fused into the DVE
  PSUM->SBUF copy as a per-partition tensor_scalar_add.

Instruction-stream scheduling (the previous version lost ~2x to PE
clock-gate throttling from dependency stalls):
  - attnV for pair p is emitted after the score matmuls of pair p+1, so
    the Exp(p) latency is hidden behind score streaming;
  - proj(b-1) is emitted after qkv(b)+v(b), giving the normalization
    chain of batch b-1's last pairs ~15us of PE work to complete;
  - psA (1-bank [128,512] tiles, bufs=4) is shared by qkv/v/proj
    accumulators and attnV outputs; psB (2-bank [128,1024], bufs=2)
    holds scores. All 8 PSUM banks in use.

PSUM discipline: two matmuls that can execute concurrently on the PE
(disjoint row groups) must never target the same PSUM bank. The only
row-disjoint concurrent writers are the quadrant-packed score matmuls,
which write different banks by construction; every other matmul spans
row 0+ and is row-serialized with its neighbors.
"""

import numpy as np
import ml_dtypes

import concourse.bass as bass
import concourse.bacc as bacc
import concourse.mybir as mybir
from concourse.tile import TileContext
from concourse.bass_utils import run_bass_kernel_spmd

f32 = mybir.dt.float32
bf16 = mybir.dt.bfloat16
i32 = mybir.dt.int32
Exp = mybir.ActivationFunctionType.Exp


B, N, C = 128, 320, 768
H, D = 12, 64
T, S = 64, 256
N_CORES = 8
BPC = B // N_CORES  # batches per core

NCT = C // 128            # 6 c-tiles of 128
NQK = 2 * C // 128        # 12 qk row-tiles
NPAIR = H // 2            # 6 head pairs
S_TILES = [(0, 128), (128, 128), (256, 64)]   # (s0, ssz) key-token tiles
SCALE = D ** -0.5
VW = 65                   # per-head V block width (64 v cols + ones)
NP_BF16 = ml_dtypes.bfloat16


def build_bass(bpc: int = BPC, n_cores: int = N_CORES, reps: int = 1):
    nc = bacc.Bacc("TRN2", target_bir_lowering=False, debug=False,
                   num_devices=n_cores)

    xt_d = nc.declare_dram_parameter("xt", [bpc, C, N], bf16, isOutput=False)
    # host-pretransposed: wqk = qkv_w[:1536].T, wv = qkv_w[1536:].T,
    # wp = proj_w.T  (all [C, m] feature-major, bf16)
    wqk_d = nc.declare_dram_parameter("wqk", [C, 2 * C], bf16, isOutput=False)
    wv_d = nc.declare_dram_parameter("wv", [C, C], bf16, isOutput=False)
    wp_d = nc.declare_dram_parameter("wp", [C, C], bf16, isOutput=False)
    # pbt[p, m] = proj_b[m*128 + p]
    pb_d = nc.declare_dram_parameter("pbt", [128, NCT], f32, isOutput=False)
    r_d = None
    if reps == 0:   # timing harness: runtime iteration count
        r_d = nc.declare_dram_parameter("reps_in", [1, 1], i32, isOutput=False)
    y_d = nc.declare_dram_parameter("y", [bpc, C, N], f32, isOutput=True)

    with TileContext(nc) as tc:
        with (
            tc.tile_pool(name="wpool", bufs=1) as wpool,
            tc.tile_pool(name="xpool", bufs=2) as xpool,
            tc.tile_pool(name="qkpool", bufs=2) as qkpool,
            tc.tile_pool(name="vpool", bufs=2) as vpool,
            tc.tile_pool(name="epool", bufs=6) as epool,
            tc.tile_pool(name="apool", bufs=2) as apool,
            tc.tile_pool(name="rpool", bufs=6) as rpool,
            tc.tile_pool(name="bpool", bufs=6) as bpool,
            tc.tile_pool(name="ypool", bufs=3) as ypool,
            tc.tile_pool(name="psA", bufs=4, space="PSUM") as psA,
            tc.tile_pool(name="psB", bufs=2, space="PSUM") as psB,
        ):
            # ---- persistent weights ----
            wqk_sb = wpool.tile([128, NCT, 2 * C], bf16)   # lhsT for q,k
            nc.sync.dma_start(out=wqk_sb[:],
                              in_=wqk_d.rearrange("(ct p) m -> p ct m", p=128))
            wv_sb = wpool.tile([128, NCT, C], bf16)        # rhs for v
            nc.sync.dma_start(out=wv_sb[:],
                              in_=wv_d.rearrange("(ct p) m -> p ct m", p=128))
            wp_sb = wpool.tile([128, NCT, C], bf16)        # lhsT for proj
            nc.sync.dma_start(out=wp_sb[:],
                              in_=wp_d.rearrange("(ct p) m -> p ct m", p=128))
            pb_sb = wpool.tile([128, NCT], f32)
            nc.sync.dma_start(out=pb_sb[:], in_=pb_d[:])
            rv = None
            if reps == 0:
                r_sb = wpool.tile([1, 1], i32)
                nc.sync.dma_start(out=r_sb[:], in_=r_d[:])
                tmp = nc.alloc_registers("reps_regs")
                nc.regs_load(tmp, r_sb[0:1, 0:1])
                rv = nc.snap(tmp, donate=True, min_val=1, max_val=4096)

            def emit_qkv(xt_sb, qk_sb):
                for m in range(NQK):
                    pacc = psA.tile([128, 512], f32, tag="pacc")
                    for ct in range(NCT):
                        nc.tensor.matmul(
                            pacc[:, 0:N],
                            wqk_sb[:, ct, m * 128:(m + 1) * 128],
                            xt_sb[:, ct, :],
                            start=(ct == 0), stop=(ct == NCT - 1))
                    nc.vector.tensor_copy(qk_sb[:, m, :], pacc[:, 0:N])

            def emit_v(xt_sb, v_sb):
                for tt, tsz in ((0, 128), (1, 128), (2, 64)):
                    for c0, csz in ((0, 512), (512, 256)):
                        pacc = psA.tile([128, 512], f32, tag="pacc")
                        for ct in range(NCT):
                            nc.tensor.matmul(
                                pacc[0:tsz, 0:csz],
                                xt_sb[:, ct, tt * 128:tt * 128 + tsz],
                                wv_sb[:, ct, c0:c0 + csz],
                                start=(ct == 0), stop=(ct == NCT - 1))
                        nh = csz // D
                        h0 = c0 // D
                        dst = v_sb[0:tsz, tt, h0 * VW:(h0 + nh) * VW] \
                            .rearrange("p (h c) -> p h c", c=VW)[:, :, 0:D]
                        src = pacc[0:tsz, 0:csz] \
                            .rearrange("p (h c) -> p h c", c=D)
                        nc.scalar.copy(dst, src)
                ones_dst = v_sb[:, :, :].rearrange(
                    "p t (h c) -> p t h c", c=VW)[:, :, :, D:VW]
                nc.vector.memset(ones_dst, 1.0)

            def emit_scores(qk_sb, p):
                """Score matmuls + Exp for pair p. Returns exp tiles."""
                mq, mk = p, NPAIR + p
                exps = []
                for st, (s0, ssz) in enumerate(S_TILES):
                    last = (st == len(S_TILES) - 1)
                    w = 320 if last else 256
                    sc = psB.tile([128, 1024], f32, tag="sc")
                    for i in (0, 1):
                        pof = 64 * i
                        nc.tensor.matmul(
                            sc[0:ssz, 512 * i:512 * i + 256],
                            qk_sb[pof:pof + 64, mk, s0:s0 + ssz],
                            qk_sb[pof:pof + 64, mq, T:N],
                            start=True, stop=True,
                            tile_position=(pof, 0))
                        if last:
                            nc.tensor.matmul(
                                sc[0:T, 512 * i + 256:512 * i + 320],
                                qk_sb[pof:pof + 64, mk, 0:T],
                                qk_sb[pof:pof + 64, mq, 0:T],
                                start=True, stop=True,
                                tile_position=(pof, 0))
                    ex = epool.tile([128, 2, 320], bf16, tag="ex")
                    gap_in = bass.AP(
                        tensor=sc.tensor, offset=sc.offset,
                        ap=[sc.ap[0], [512, 2], [1, w]])
                    nc.scalar.activation(out=ex[0:ssz, :, 0:w],
                                         in_=gap_in[0:ssz],
                                         func=Exp, scale=SCALE)
                    exps.append(ex)
                return exps

            def emit_av_norm(v_sb, attn_sb, p, exps):
                """attn @ [v | 1] + normalization for pair p.

                The whole O tile is copied PSUM->SBUF on ACT right after
                the matmuls: this frees the psA buffer ~1us after the
                matmul stop (instead of ~3us later at the DVE multiply),
                so the next batch's qkv never stalls on psA recycling;
                it also feeds the custom DVE reciprocal from SBUF (its
                bitwise seed misreads PSUM) at the same ACT cost as a
                separate denominator-row copy."""
                for i, h in enumerate((2 * p, 2 * p + 1)):
                    O = psA.tile([128, 512], f32, tag="pacc")
                    for st, (s0, ssz) in enumerate(S_TILES):
                        nc.tensor.matmul(
                            O[0:VW, T:N],
                            v_sb[0:ssz, st, h * VW:(h + 1) * VW],
                            exps[st][0:ssz, i, 0:256],
                            start=(st == 0),
                            stop=(st == len(S_TILES) - 1))
                    nc.tensor.matmul(
                        O[0:VW, 0:T],
                        v_sb[0:T, 0, h * VW:(h + 1) * VW],
                        exps[2][0:T, i, 256:320],
                        start=True, stop=True)

                    # ACT copies the denominator row to SBUF partition 0
                    # (the custom DVE reciprocal misreads PSUM, and also
                    # mishandles non-zero partition offsets), then the
                    # ~5x-faster approx reciprocal runs on DVE.
                    den = rpool.tile([1, N], f32, tag="den")
                    nc.scalar.copy(den[0:1, :], O[64:65, 0:N])
                    rec = rpool.tile([1, N], f32, tag="rec")
                    nc.vector.reciprocal_approx_fast(out=rec[0:1, :],
                                                     in_=den[0:1, :])
                    rb = bpool.tile([64, N], f32, tag="rb")
                    nc.gpsimd.partition_broadcast(rb[0:64, :], rec[0:1, :])
                    nc.vector.tensor_mul(
                        attn_sb[64 * i:64 * i + 64, p, :],
                        O[0:64, 0:N], rb[0:64, :])

            def emit_proj_m(attn_sb, b, m):
                yp = psA.tile([128, 512], f32, tag="pacc")
                for ct in range(NCT):
                    nc.tensor.matmul(
                        yp[:, 0:N],
                        wp_sb[:, ct, m * 128:(m + 1) * 128],
                        attn_sb[:, ct, :],
                        start=(ct == 0), stop=(ct == NCT - 1))
                yt_sb = ypool.tile([128, N], f32, tag="yt")
                nc.vector.tensor_scalar_add(yt_sb[:], yp[:, 0:N],
                                            pb_sb[:, m:m + 1])
                nc.sync.dma_start(out=y_d[b, m * 128:(m + 1) * 128, :],
                                  in_=yt_sb[:])

            def body(_iv=None):
                prev = None           # (attn_sb, b) awaiting projection
                for b in range(bpc):
                    xt_sb = xpool.tile([128, NCT, N], bf16)
                    nc.sync.dma_start(
                        out=xt_sb[:],
                        in_=xt_d[b].rearrange("(ct p) t -> p ct t", p=128))

                    qk_sb = qkpool.tile([128, NQK, N], bf16)
                    emit_qkv(xt_sb, qk_sb)
                    v_sb = vpool.tile([128, 3, H * VW], bf16)
                    emit_v(xt_sb, v_sb)

                    # proj(b-1) m-tiles are interleaved into the attention
                    # pair slots below: their long 320-free streams keep
                    # the PE activity window busy through the short-stream
                    # attention phase (HAM clock-gate stays at 8/8), and
                    # the norm chain of b-1's tail pairs has had qkv+v(b)
                    # to complete.
                    # proj m4/m5 are emitted AFTER the last attnV so the
                    # next batch's first qkv psA allocations recycle the
                    # fast-freed yp tiles rather than the tail pairs' O
                    # tiles (whose norm chains finish ~2.5us later).
                    attn_sb = apool.tile([128, NPAIR, N], bf16)
                    pend = None       # (p, exps) with attnV not yet emitted
                    for p in range(NPAIR):
                        exps = emit_scores(qk_sb, p)
                        if pend is not None:
                            emit_av_norm(v_sb, attn_sb, *pend)
                        if prev is not None and p < 4:
                            emit_proj_m(*prev, m=p)
                        pend = (p, exps)
                    emit_av_norm(v_sb, attn_sb, *pend)
                    if prev is not None:
                        emit_proj_m(*prev, m=4)
                        emit_proj_m(*prev, m=5)
                    prev = (attn_sb, b)
                for m in range(NCT):
                    emit_proj_m(*prev, m=m)

            if reps == 1:
                body()
            elif reps == 0:
                with tc.For_i(0, rv, 1) as _i:
                    body(_i)
            else:
                with tc.For_i(0, reps, 1) as _i:
                    body(_i)

    nc.compile()
    return nc


_NC_CACHE = {}


def _get_nc(bpc: int = BPC):
    if bpc not in _NC_CACHE:
        _NC_CACHE[bpc] = build_bass(bpc)
    return _NC_CACHE[bpc]


def make_in_maps(x1, x2, qkv_w, proj_w, proj_b, n_cores=N_CORES):
    x1 = np.asarray(x1, dtype=np.float32)
    x2 = np.asarray(x2, dtype=np.float32)
    qkv_w = np.asarray(qkv_w, dtype=np.float32)
    proj_w = np.asarray(proj_w, dtype=np.float32)
    proj_b = np.asarray(proj_b, dtype=np.float32)

    b = x1.shape[0]
    xt = np.empty((b, C, N), dtype=NP_BF16)
    xt[:, :, :T] = x1[:, :T, :].transpose(0, 2, 1).astype(NP_BF16)
    xt[:, :, T:] = x2[:, T:, :].transpose(0, 2, 1).astype(NP_BF16)

    wqk = np.ascontiguousarray(qkv_w[:2 * C].T).astype(NP_BF16)
    wv = np.ascontiguousarray(qkv_w[2 * C:].T).astype(NP_BF16)
    wp = np.ascontiguousarray(proj_w.T).astype(NP_BF16)
    pbt = np.ascontiguousarray(proj_b.reshape(NCT, 128).T)  # [128, NCT] f32

    bpc = b // n_cores
    return [
        {"xt": xt[c * bpc:(c + 1) * bpc], "wqk": wqk, "wv": wv, "wp": wp,
         "pbt": pbt}
        for c in range(n_cores)
    ], bpc


def kernel(x1, x2, qkv_w, proj_w, proj_b):
    in_maps, bpc = make_in_maps(x1, x2, qkv_w, proj_w, proj_b)
    nc = _get_nc(bpc)
    res = run_bass_kernel_spmd(nc, in_maps, list(range(N_CORES)))
    yt = np.concatenate([res.results[c]["y"] for c in range(N_CORES)], axis=0)
    return np.ascontiguousarray(yt.transpose(0, 2, 1))


# revision 19
# speedup vs baseline: 1.1683x; 1.1666x over previous
"""Sparse attention (template/search) Trainium2 Bass kernel.

Problem: B=128, N=320 (T=64 template + S=256 search), C=768, H=12, d=64.
  x = concat(x1[:, :64], x2[:, 64:])
  qkv = x @ qkv_w.T ; per-head attention (template->template, search->all)
  out = attn @ proj_w.T + proj_b
Pure data parallel over batch: 16 batches per core on 8 cores.

Dataflow (per batch, all feature-major / "transposed" on chip, bf16
matmul operands, f32 PSUM accumulation):
  xT [C, N] --wqk--> qkT [2C rows, N]      (12 m-tiles, free dim 320)
  xT --wv--> v token-major per-head 65-wide blocks ([64 v cols | ones])
  scores^T [s, t] per head pair, quadrant-packed (two heads at
  tile_position row offsets 0/64 writing different PSUM banks so they
  stream concurrently); template scores ride the spare columns of the
  third s-tile.  Exp (ACT) -> bf16 tiles; attn @ [v|1] puts the softmax
  numerator in rows 0..63 and the denominator in row 64 of a psA tile.
  Normalization: ACT copies the denominator row to SBUF, DVE
  reciprocal_approx_fast (NOT the 8-cycle/elem iterative divide),
  GpSimd partition-broadcast, DVE multiply -> attn bf16.
  Projection reads attn feature-major; the bias is fused into the DVE
  PSUM->SBUF copy as a per-partition tensor_scalar_add.

Instruction-stream scheduling (the previous version lost ~2x to PE
clock-gate throttling from dependency stalls):
  - attnV for pair p is emitted after the score matmuls of pair p+1, so
    the Exp(p) latency is hidden behind score streaming;
  - proj(b-1) is emitted after qkv(b)+v(b), giving the normalization
    chain of batch b-1's last pairs ~15us of PE work to complete;
  - psA (1-bank [128,512] tiles, bufs=4) is shared by qkv/v/proj
    accumulators and attnV outputs; psB (2-bank [128,1024], bufs=2)
    holds scores. All 8 PSUM banks in use.

PSUM discipline: two matmuls that can execute concurrently on the PE
(disjoint row groups) must never target the same PSUM bank. The only
row-disjoint concurrent writers are the quadrant-packed score matmuls,
which write different banks by construction; every other matmul spans
row 0+ and is row-serialized with its neighbors.
"""

import numpy as np
import ml_dtypes

import concourse.bass as bass
import concourse.bacc as bacc
import concourse.mybir as mybir
from concourse.tile import TileContext
from concourse.bass_utils import run_bass_kernel_spmd

f32 = mybir.dt.float32
bf16 = mybir.dt.bfloat16
i32 = mybir.dt.int32
Exp = mybir.ActivationFunctionType.Exp


B, N, C = 128, 320, 768
H, D = 12, 64
T, S = 64, 256
N_CORES = 8
BPC = B // N_CORES  # batches per core

NCT = C // 128            # 6 c-tiles of 128
NQK = 2 * C // 128        # 12 qk row-tiles
NPAIR = H // 2            # 6 head pairs
S_TILES = [(0, 128), (128, 128), (256, 64)]   # (s0, ssz) key-token tiles
SCALE = D ** -0.5
VW = 65                   # per-head V block width (64 v cols + ones)
NP_BF16 = ml_dtypes.bfloat16


def build_bass(bpc: int = BPC, n_cores: int = N_CORES, reps: int = 1):
    nc = bacc.Bacc("TRN2", target_bir_lowering=False, debug=False,
                   num_devices=n_cores)

    xt_d = nc.declare_dram_parameter("xt", [bpc, C, N], bf16, isOutput=False)
    # host-pretransposed: wqk = qkv_w[:1536].T, wv = qkv_w[1536:].T,
    # wp = proj_w.T  (all [C, m] feature-major, bf16)
    wqk_d = nc.declare_dram_parameter("wqk", [C, 2 * C], bf16, isOutput=False)
    wv_d = nc.declare_dram_parameter("wv", [C, C], bf16, isOutput=False)
    wp_d = nc.declare_dram_parameter("wp", [C, C], bf16, isOutput=False)
    # pbt[p, m] = proj_b[m*128 + p]
    pb_d = nc.declare_dram_parameter("pbt", [128, NCT], f32, isOutput=False)
    r_d = None
    if reps == 0:   # timing harness: runtime iteration count
        r_d = nc.declare_dram_parameter("reps_in", [1, 1], i32, isOutput=False)
    y_d = nc.declare_dram_parameter("y", [bpc, C, N], f32, isOutput=True)

    with TileContext(nc) as tc:
        with (
            tc.tile_pool(name="wpool", bufs=1) as wpool,
            tc.tile_pool(name="xpool", bufs=2) as xpool,
            tc.tile_pool(name="qkpool", bufs=2) as qkpool,
            tc.tile_pool(name="vpool", bufs=2) as vpool,
            tc.tile_pool(name="epool", bufs=6) as epool,
            tc.tile_pool(name="apool", bufs=2) as apool,
            tc.tile_pool(name="rpool", bufs=6) as rpool,
            tc.tile_pool(name="opool", bufs=6) as opool,
            tc.tile_pool(name="bpool", bufs=6) as bpool,
            tc.tile_pool(name="ypool", bufs=3) as ypool,
            tc.tile_pool(name="psA", bufs=4, space="PSUM") as psA,
            tc.tile_pool(name="psB", bufs=2, space="PSUM") as psB,
        ):
            # ---- persistent weights ----
            wqk_sb = wpool.tile([128, NCT, 2 * C], bf16)   # lhsT for q,k
            nc.sync.dma_start(out=wqk_sb[:],
                              in_=wqk_d.rearrange("(ct p) m -> p ct m", p=128))
            wv_sb = wpool.tile([128, NCT, C], bf16)        # rhs for v
            nc.sync.dma_start(out=wv_sb[:],
                              in_=wv_d.rearrange("(ct p) m -> p ct m", p=128))
            wp_sb = wpool.tile([128, NCT, C], bf16)        # lhsT for proj
            nc.sync.dma_start(out=wp_sb[:],
                              in_=wp_d.rearrange("(ct p) m -> p ct m", p=128))
            pb_sb = wpool.tile([128, NCT], f32)
            nc.sync.dma_start(out=pb_sb[:], in_=pb_d[:])
            rv = None
            if reps == 0:
                r_sb = wpool.tile([1, 1], i32)
                nc.sync.dma_start(out=r_sb[:], in_=r_d[:])
                tmp = nc.alloc_registers("reps_regs")
                nc.regs_load(tmp, r_sb[0:1, 0:1])
                rv = nc.snap(tmp, donate=True, min_val=1, max_val=4096)

            def emit_qkv(xt_sb, qk_sb):
                for m in range(NQK):
                    pacc = psA.tile([128, 512], f32, tag="pacc")
                    for ct in range(NCT):
                        nc.tensor.matmul(
                            pacc[:, 0:N],
                            wqk_sb[:, ct, m * 128:(m + 1) * 128],
                            xt_sb[:, ct, :],
                            start=(ct == 0), stop=(ct == NCT - 1))
                    nc.vector.tensor_copy(qk_sb[:, m, :], pacc[:, 0:N])

            def emit_v(xt_sb, v_sb):
                for tt, tsz in ((0, 128), (1, 128), (2, 64)):
                    for c0, csz in ((0, 512), (512, 256)):
                        pacc = psA.tile([128, 512], f32, tag="pacc")
                        for ct in range(NCT):
                            nc.tensor.matmul(
                                pacc[0:tsz, 0:csz],
                                xt_sb[:, ct, tt * 128:tt * 128 + tsz],
                                wv_sb[:, ct, c0:c0 + csz],
                                start=(ct == 0), stop=(ct == NCT - 1))
                        nh = csz // D
                        h0 = c0 // D
                        dst = v_sb[0:tsz, tt, h0 * VW:(h0 + nh) * VW] \
                            .rearrange("p (h c) -> p h c", c=VW)[:, :, 0:D]
                        src = pacc[0:tsz, 0:csz] \
                            .rearrange("p (h c) -> p h c", c=D)
                        nc.vector.tensor_copy(dst, src)
                ones_dst = v_sb[:, :, :].rearrange(
                    "p t (h c) -> p t h c", c=VW)[:, :, :, D:VW]
                nc.vector.memset(ones_dst, 1.0)

            def emit_scores(qk_sb, p):
                """Score matmuls + Exp for pair p. Returns exp tiles."""
                mq, mk = p, NPAIR + p
                exps = []
                for st, (s0, ssz) in enumerate(S_TILES):
                    last = (st == len(S_TILES) - 1)
                    w = 320 if last else 256
                    sc = psB.tile([128, 1024], f32, tag="sc")
                    for i in (0, 1):
                        pof = 64 * i
                        nc.tensor.matmul(
                            sc[0:ssz, 512 * i:512 * i + 256],
                            qk_sb[pof:pof + 64, mk, s0:s0 + ssz],
                            qk_sb[pof:pof + 64, mq, T:N],
                            start=True, stop=True,
                            tile_position=(pof, 0))
                        if last:
                            nc.tensor.matmul(
                                sc[0:T, 512 * i + 256:512 * i + 320],
                                qk_sb[pof:pof + 64, mk, 0:T],
                                qk_sb[pof:pof + 64, mq, 0:T],
                                start=True, stop=True,
                                tile_position=(pof, 0))
                    ex = epool.tile([128, 2, 320], bf16, tag="ex")
                    gap_in = bass.AP(
                        tensor=sc.tensor, offset=sc.offset,
                        ap=[sc.ap[0], [512, 2], [1, w]])
                    nc.scalar.activation(out=ex[0:ssz, :, 0:w],
                                         in_=gap_in[0:ssz],
                                         func=Exp, scale=SCALE)
                    exps.append(ex)
                return exps

            def emit_av_norm(v_sb, attn_sb, p, exps):
                """attn @ [v | 1] + normalization for pair p.

                The whole O tile is copied PSUM->SBUF on ACT right after
                the matmuls: this frees the psA buffer ~1us after the
                matmul stop (instead of ~3us later at the DVE multiply),
                so the next batch's qkv never stalls on psA recycling;
                it also feeds the custom DVE reciprocal from SBUF (its
                bitwise seed misreads PSUM) at the same ACT cost as a
                separate denominator-row copy."""
                for i, h in enumerate((2 * p, 2 * p + 1)):
                    O = psA.tile([128, 512], f32, tag="pacc")
                    for st, (s0, ssz) in enumerate(S_TILES):
                        nc.tensor.matmul(
                            O[0:VW, T:N],
                            v_sb[0:ssz, st, h * VW:(h + 1) * VW],
                            exps[st][0:ssz, i, 0:256],
                            start=(st == 0),
                            stop=(st == len(S_TILES) - 1))
                    nc.tensor.matmul(
                        O[0:VW, 0:T],
                        v_sb[0:T, 0, h * VW:(h + 1) * VW],
                        exps[2][0:T, i, 256:320],
                        start=True, stop=True)

                    # ACT copies numerator rows and the denominator row to
                    # SBUF back-to-back; O (psA) is freed ~1us after the
                    # matmuls instead of ~3us later at the DVE multiply,
                    # so the interleaved-proj psA FIFO never stalls. The
                    # custom DVE reciprocal reads the partition-0 den tile
                    # (it misreads PSUM and non-zero partition offsets).
                    ocp = opool.tile([64, N], f32, tag="ocp")
                    nc.scalar.copy(ocp[0:64, :], O[0:64, 0:N])
                    den = rpool.tile([1, N], f32, tag="den")
                    nc.scalar.copy(den[0:1, :], O[64:65, 0:N])
                    rec = rpool.tile([1, N], f32, tag="rec")
                    nc.vector.reciprocal_approx_fast(out=rec[0:1, :],
                                                     in_=den[0:1, :])
                    rb = bpool.tile([64, N], f32, tag="rb")
                    nc.gpsimd.partition_broadcast(rb[0:64, :], rec[0:1, :])
                    nc.vector.tensor_mul(
                        attn_sb[64 * i:64 * i + 64, p, :],
                        ocp[0:64, :], rb[0:64, :])

            def emit_proj_m(attn_sb, b, m):
                yp = psA.tile([128, 512], f32, tag="pacc")
                for ct in range(NCT):
                    nc.tensor.matmul(
                        yp[:, 0:N],
                        wp_sb[:, ct, m * 128:(m + 1) * 128],
                        attn_sb[:, ct, :],
                        start=(ct == 0), stop=(ct == NCT - 1))
                yt_sb = ypool.tile([128, N], f32, tag="yt")
                nc.vector.tensor_scalar_add(yt_sb[:], yp[:, 0:N],
                                            pb_sb[:, m:m + 1])
                nc.sync.dma_start(out=y_d[b, m * 128:(m + 1) * 128, :],
                                  in_=yt_sb[:])

            def body(_iv=None):
                prev = None           # (attn_sb, b) awaiting projection
                for b in range(bpc):
                    xt_sb = xpool.tile([128, NCT, N], bf16)
                    nc.sync.dma_start(
                        out=xt_sb[:],
                        in_=xt_d[b].rearrange("(ct p) t -> p ct t", p=128))

                    qk_sb = qkpool.tile([128, NQK, N], bf16)
                    emit_qkv(xt_sb, qk_sb)
                    v_sb = vpool.tile([128, 3, H * VW], bf16)
                    emit_v(xt_sb, v_sb)

                    # proj(b-1) m-tiles are interleaved into the attention
                    # pair slots below: their long 320-free streams keep
                    # the PE activity window busy through the short-stream
                    # attention phase (HAM clock-gate stays at 8/8), and
                    # the norm chain of b-1's tail pairs has had qkv+v(b)
                    # to complete.
                    # proj m4/m5 are emitted AFTER the last attnV so the
                    # next batch's first qkv psA allocations recycle the
                    # fast-freed yp tiles rather than the tail pairs' O
                    # tiles (whose norm chains finish ~2.5us later).
                    attn_sb = apool.tile([128, NPAIR, N], bf16)
                    pend = None       # (p, exps) with attnV not yet emitted
                    for p in range(NPAIR):
                        exps = emit_scores(qk_sb, p)
                        if pend is not None:
                            emit_av_norm(v_sb, attn_sb, *pend)
                        if prev is not None and p < 4:
                            emit_proj_m(*prev, m=p)
                        pend = (p, exps)
                    emit_av_norm(v_sb, attn_sb, *pend)
                    if prev is not None:
                        emit_proj_m(*prev, m=4)
                        emit_proj_m(*prev, m=5)
                    prev = (attn_sb, b)
                for m in range(NCT):
                    emit_proj_m(*prev, m=m)

            if reps == 1:
                body()
            elif reps == 0:
                with tc.For_i(0, rv, 1) as _i:
                    body(_i)
            else:
                with tc.For_i(0, reps, 1) as _i:
                    body(_i)

    nc.compile()
    return nc


_NC_CACHE = {}


def _get_nc(bpc: int = BPC):
    if bpc not in _NC_CACHE:
        _NC_CACHE[bpc] = build_bass(bpc)
    return _NC_CACHE[bpc]


def make_in_maps(x1, x2, qkv_w, proj_w, proj_b, n_cores=N_CORES):
    x1 = np.asarray(x1, dtype=np.float32)
    x2 = np.asarray(x2, dtype=np.float32)
    qkv_w = np.asarray(qkv_w, dtype=np.float32)
    proj_w = np.asarray(proj_w, dtype=np.float32)
    proj_b = np.asarray(proj_b, dtype=np.float32)

    b = x1.shape[0]
    xt = np.empty((b, C, N), dtype=NP_BF16)
    xt[:, :, :T] = x1[:, :T, :].transpose(0, 2, 1).astype(NP_BF16)
    xt[:, :, T:] = x2[:, T:, :].transpose(0, 2, 1).astype(NP_BF16)

    wqk = np.ascontiguousarray(qkv_w[:2 * C].T).astype(NP_BF16)
    wv = np.ascontiguousarray(qkv_w[2 * C:].T).astype(NP_BF16)
    wp = np.ascontiguousarray(proj_w.T).astype(NP_BF16)
    pbt = np.ascontiguousarray(proj_b.reshape(NCT, 128).T)  # [128, NCT] f32

    bpc = b // n_cores
    return [
        {"xt": xt[c * bpc:(c + 1) * bpc], "wqk": wqk, "wv": wv, "wp": wp,
         "pbt": pbt}
        for c in range(n_cores)
    ], bpc


def kernel(x1, x2, qkv_w, proj_w, proj_b):
    in_maps, bpc = make_in_maps(x1, x2, qkv_w, proj_w, proj_b)
    nc = _get_nc(bpc)
    res = run_bass_kernel_spmd(nc, in_maps, list(range(N_CORES)))
    yt = np.concatenate([res.results[c]["y"] for c in range(N_CORES)], axis=0)
    return np.ascontiguousarray(yt.transpose(0, 2, 1))


# revision 20
# speedup vs baseline: 1.2051x; 1.0315x over previous
"""Sparse attention (template/search) Trainium2 Bass kernel.

Problem: B=128, N=320 (T=64 template + S=256 search), C=768, H=12, d=64.
  x = concat(x1[:, :64], x2[:, 64:])
  qkv = x @ qkv_w.T ; per-head attention (template->template, search->all)
  out = attn @ proj_w.T + proj_b
Pure data parallel over batch: 16 batches per core on 8 cores.

Dataflow (per batch, all feature-major / "transposed" on chip, bf16
matmul operands, f32 PSUM accumulation):
  xT [C, N] --wqk--> qkT [2C rows, N]      (12 m-tiles, free dim 320)
  xT --wv--> v token-major per-head 65-wide blocks ([64 v cols | ones])
  scores^T [s, t] per head pair, quadrant-packed (two heads at
  tile_position row offsets 0/64 writing different PSUM banks so they
  stream concurrently); template scores ride the spare columns of the
  third s-tile.  Exp (ACT) -> bf16 tiles; attn @ [v|1] puts the softmax
  numerator in rows 0..63 and the denominator in row 64 of a psA tile.
  Normalization: DVE copies the denominator row to SBUF partition 0
  (the custom reciprocal misreads PSUM and non-zero partition offsets),
  DVE reciprocal_approx_fast (NOT the 8-cycle/elem iterative divide),
  GpSimd partition-broadcast, DVE multiply -> attn bf16.  The proj bias
  is fused into the DVE PSUM->SBUF copy as a tensor_scalar_add.

Scheduling: the whole kernel is software-pipelined at depth 2 in
uniform slots -- window b emits, per slot p in 0..5:
  [qkv m-tiles 2p,2p+1 of batch b] [scores pair p of batch b-1]
  [attnV+norm pair p-1 of b-1] [proj m-tile p-1 of batch b-2]
with batch b's v-tiles and the attention/proj stragglers in the window
tail.  This keeps the PE's activity window saturated with long streams
(the HAM clock gate stays at 8/8 = 2.4GHz) and spreads the attention
phase's ACT/DVE/GpSimd work (Exp, denominator copies, reciprocals,
normalization multiplies) over the full window instead of cramming it
into a short attention phase where it stalls the PE.  Engine budget per
window (~40us): ACT = Exps + v copies ~18us, DVE = qk copies + den +
recip + mul + yt ~27us, GpSimd = broadcasts ~9us, all under the PE.

PSUM discipline: two matmuls that can execute concurrently on the PE
(disjoint row groups) must never target the same PSUM bank.  The only
row-disjoint concurrent writers are the quadrant-packed score matmuls,
which write different banks by construction; every other matmul spans
row 0+ and is row-serialized with its neighbors.  psA (1-bank [128,512]
tiles, bufs=4) rotates qkv/v/proj accumulators and attnV outputs; psB
(2-bank [128,1024], bufs=2) holds scores; all 8 banks in use.
"""

import numpy as np
import ml_dtypes

import concourse.bass as bass
import concourse.bacc as bacc
import concourse.mybir as mybir
from concourse.tile import TileContext
from concourse.bass_utils import run_bass_kernel_spmd

f32 = mybir.dt.float32
bf16 = mybir.dt.bfloat16
i32 = mybir.dt.int32
Exp = mybir.ActivationFunctionType.Exp


B, N, C = 128, 320, 768
H, D = 12, 64
T, S = 64, 256
N_CORES = 8
BPC = B // N_CORES  # batches per core

NCT = C // 128            # 6 c-tiles of 128
NQK = 2 * C // 128        # 12 qk row-tiles
NPAIR = H // 2            # 6 head pairs
S_TILES = [(0, 128), (128, 128), (256, 64)]   # (s0, ssz) key-token tiles
SCALE = D ** -0.5
VW = 65                   # per-head V block width (64 v cols + ones)
V_GROUPS = [(0, 128, 0, 512), (0, 128, 512, 256),
            (1, 128, 0, 512), (1, 128, 512, 256),
            (2, 64, 0, 512), (2, 64, 512, 256)]  # (tt, tsz, c0, csz)
NP_BF16 = ml_dtypes.bfloat16


def build_bass(bpc: int = BPC, n_cores: int = N_CORES, reps: int = 1):
    nc = bacc.Bacc("TRN2", target_bir_lowering=False, debug=False,
                   num_devices=n_cores)

    xt_d = nc.declare_dram_parameter("xt", [bpc, C, N], bf16, isOutput=False)
    # host-pretransposed: wqk = qkv_w[:1536].T, wv = qkv_w[1536:].T,
    # wp = proj_w.T  (all [C, m] feature-major, bf16)
    wqk_d = nc.declare_dram_parameter("wqk", [C, 2 * C], bf16, isOutput=False)
    wv_d = nc.declare_dram_parameter("wv", [C, C], bf16, isOutput=False)
    wp_d = nc.declare_dram_parameter("wp", [C, C], bf16, isOutput=False)
    # pbt[p, m] = proj_b[m*128 + p]
    pb_d = nc.declare_dram_parameter("pbt", [128, NCT], f32, isOutput=False)
    r_d = None
    if reps == 0:   # timing harness: runtime iteration count
        r_d = nc.declare_dram_parameter("reps_in", [1, 1], i32, isOutput=False)
    y_d = nc.declare_dram_parameter("y", [bpc, C, N], f32, isOutput=True)

    with TileContext(nc) as tc:
        with (
            tc.tile_pool(name="wpool", bufs=1) as wpool,
            tc.tile_pool(name="xpool", bufs=3) as xpool,
            tc.tile_pool(name="qkpool", bufs=2) as qkpool,
            tc.tile_pool(name="vpool", bufs=2) as vpool,
            tc.tile_pool(name="epool", bufs=6) as epool,
            tc.tile_pool(name="apool", bufs=3) as apool,
            tc.tile_pool(name="rpool", bufs=8) as rpool,
            tc.tile_pool(name="bpool", bufs=6) as bpool,
            tc.tile_pool(name="ypool", bufs=3) as ypool,
            tc.tile_pool(name="psA", bufs=4, space="PSUM") as psA,
            tc.tile_pool(name="psB", bufs=2, space="PSUM") as psB,
        ):
            # ---- persistent weights ----
            wqk_sb = wpool.tile([128, NCT, 2 * C], bf16)   # lhsT for q,k
            nc.sync.dma_start(out=wqk_sb[:],
                              in_=wqk_d.rearrange("(ct p) m -> p ct m", p=128))
            wv_sb = wpool.tile([128, NCT, C], bf16)        # rhs for v
            nc.sync.dma_start(out=wv_sb[:],
                              in_=wv_d.rearrange("(ct p) m -> p ct m", p=128))
            wp_sb = wpool.tile([128, NCT, C], bf16)        # lhsT for proj
            nc.sync.dma_start(out=wp_sb[:],
                              in_=wp_d.rearrange("(ct p) m -> p ct m", p=128))
            pb_sb = wpool.tile([128, NCT], f32)
            nc.sync.dma_start(out=pb_sb[:], in_=pb_d[:])
            rv = None
            if reps == 0:
                r_sb = wpool.tile([1, 1], i32)
                nc.sync.dma_start(out=r_sb[:], in_=r_d[:])
                tmp = nc.alloc_registers("reps_regs")
                nc.regs_load(tmp, r_sb[0:1, 0:1])
                rv = nc.snap(tmp, donate=True, min_val=1, max_val=4096)

            def emit_qkv_m(xt_sb, qk_sb, m):
                pacc = psA.tile([128, 512], f32, tag="pacc")
                for ct in range(NCT):
                    nc.tensor.matmul(
                        pacc[:, 0:N],
                        wqk_sb[:, ct, m * 128:(m + 1) * 128],
                        xt_sb[:, ct, :],
                        start=(ct == 0), stop=(ct == NCT - 1))
                nc.vector.tensor_copy(qk_sb[:, m, :], pacc[:, 0:N])

            def emit_v_g(xt_sb, v_sb, g):
                tt, tsz, c0, csz = V_GROUPS[g]
                pacc = psA.tile([128, 512], f32, tag="pacc")
                for ct in range(NCT):
                    nc.tensor.matmul(
                        pacc[0:tsz, 0:csz],
                        xt_sb[:, ct, tt * 128:tt * 128 + tsz],
                        wv_sb[:, ct, c0:c0 + csz],
                        start=(ct == 0), stop=(ct == NCT - 1))
                nh = csz // D
                h0 = c0 // D
                dst = v_sb[0:tsz, tt, h0 * VW:(h0 + nh) * VW] \
                    .rearrange("p (h c) -> p h c", c=VW)[:, :, 0:D]
                src = pacc[0:tsz, 0:csz].rearrange("p (h c) -> p h c", c=D)
                nc.scalar.copy(dst, src)

            def emit_v_ones(v_sb):
                ones_dst = v_sb[:, :, :].rearrange(
                    "p t (h c) -> p t h c", c=VW)[:, :, :, D:VW]
                nc.vector.memset(ones_dst, 1.0)

            def emit_sc(qk_sb, p):
                """Score matmuls + Exp for pair p. Returns exp tiles."""
                mq, mk = p, NPAIR + p
                exps = []
                for st, (s0, ssz) in enumerate(S_TILES):
                    last = (st == len(S_TILES) - 1)
                    w = 320 if last else 256
                    sc = psB.tile([128, 1024], f32, tag="sc")
                    for i in (0, 1):
                        pof = 64 * i
                        nc.tensor.matmul(
                            sc[0:ssz, 512 * i:512 * i + 256],
                            qk_sb[pof:pof + 64, mk, s0:s0 + ssz],
                            qk_sb[pof:pof + 64, mq, T:N],
                            start=True, stop=True,
                            tile_position=(pof, 0))
                        if last:
                            nc.tensor.matmul(
                                sc[0:T, 512 * i + 256:512 * i + 320],
                                qk_sb[pof:pof + 64, mk, 0:T],
                                qk_sb[pof:pof + 64, mq, 0:T],
                                start=True, stop=True,
                                tile_position=(pof, 0))
                    ex = epool.tile([128, 2, 320], bf16, tag="ex")
                    gap_in = bass.AP(
                        tensor=sc.tensor, offset=sc.offset,
                        ap=[sc.ap[0], [512, 2], [1, w]])
                    nc.scalar.activation(out=ex[0:ssz, :, 0:w],
                                         in_=gap_in[0:ssz],
                                         func=Exp, scale=SCALE)
                    exps.append(ex)
                return exps

            def emit_av(v_sb, attn_sb, p, exps):
                """attn @ [v | 1] + normalization for pair p."""
                for i, h in enumerate((2 * p, 2 * p + 1)):
                    O = psA.tile([128, 512], f32, tag="pacc")
                    for st, (s0, ssz) in enumerate(S_TILES):
                        nc.tensor.matmul(
                            O[0:VW, T:N],
                            v_sb[0:ssz, st, h * VW:(h + 1) * VW],
                            exps[st][0:ssz, i, 0:256],
                            start=(st == 0),
                            stop=(st == len(S_TILES) - 1))
                    nc.tensor.matmul(
                        O[0:VW, 0:T],
                        v_sb[0:T, 0, h * VW:(h + 1) * VW],
                        exps[2][0:T, i, 256:320],
                        start=True, stop=True)

                    den = rpool.tile([1, N], f32, tag="den")
                    nc.vector.tensor_copy(den[0:1, :], O[64:65, 0:N])
                    rec = rpool.tile([1, N], f32, tag="rec")
                    nc.vector.reciprocal_approx_fast(out=rec[0:1, :],
                                                     in_=den[0:1, :])
                    rb = bpool.tile([64, N], f32, tag="rb")
                    nc.gpsimd.partition_broadcast(rb[0:64, :], rec[0:1, :])
                    nc.vector.tensor_mul(
                        attn_sb[64 * i:64 * i + 64, p, :],
                        O[0:64, 0:N], rb[0:64, :])

            def emit_pj(attn_sb, b, m):
                yp = psA.tile([128, 512], f32, tag="pacc")
                for ct in range(NCT):
                    nc.tensor.matmul(
                        yp[:, 0:N],
                        wp_sb[:, ct, m * 128:(m + 1) * 128],
                        attn_sb[:, ct, :],
                        start=(ct == 0), stop=(ct == NCT - 1))
                yt_sb = ypool.tile([128, N], f32, tag="yt")
                nc.vector.tensor_scalar_add(yt_sb[:], yp[:, 0:N],
                                            pb_sb[:, m:m + 1])
                nc.sync.dma_start(out=y_d[b, m * 128:(m + 1) * 128, :],
                                  in_=yt_sb[:])

            def body(_iv=None):
                xts = {}

                def get_xt(b):
                    if b not in xts and 0 <= b < bpc:
                        t = xpool.tile([128, NCT, N], bf16, name="xt_sb")
                        nc.sync.dma_start(
                            out=t[:],
                            in_=xt_d[b].rearrange("(ct p) t -> p ct t", p=128))
                        xts[b] = t
                    return xts.get(b)

                get_xt(0)
                pend = None     # (qk_sb, v_sb, b): attention runs next window
                projq = []      # [(attn_sb, b)] awaiting projection

                def attn_window(xt_sb, qk_sb, v_sb):
                    """One pipelined window: qkv/v of the current batch
                    (None for the drain window) + attention of pend +
                    projection of projq[0]."""
                    pqk, pv, pb_ = pend
                    attn_sb = apool.tile([128, NPAIR, N], bf16)
                    pj = projq.pop(0) if len(projq) > 0 else None
                    exps = {}
                    for p in range(NPAIR):
                        if qk_sb is not None:
                            emit_qkv_m(xt_sb, qk_sb, 2 * p)
                            emit_qkv_m(xt_sb, qk_sb, 2 * p + 1)
                        exps[p] = emit_sc(pqk, p)
                        if p >= 1:
                            emit_av(pv, attn_sb, p - 1, exps.pop(p - 1))
                            if pj is not None:
                                emit_pj(*pj, m=p - 1)
                    if v_sb is not None:
                        emit_v_g(xt_sb, v_sb, 0)
                        emit_v_g(xt_sb, v_sb, 1)
                    emit_av(pv, attn_sb, NPAIR - 1, exps.pop(NPAIR - 1))
                    if v_sb is not None:
                        emit_v_g(xt_sb, v_sb, 2)
                        emit_v_g(xt_sb, v_sb, 3)
                    if pj is not None:
                        emit_pj(*pj, m=NPAIR - 1)
                    if v_sb is not None:
                        emit_v_g(xt_sb, v_sb, 4)
                        emit_v_g(xt_sb, v_sb, 5)
                        emit_v_ones(v_sb)
                    projq.append((attn_sb, pb_))

                for b in range(bpc):
                    xt_sb = get_xt(b)
                    get_xt(b + 1)   # prefetch next batch's input early
                    qk_sb = qkpool.tile([128, NQK, N], bf16)
                    v_sb = vpool.tile([128, 3, H * VW], bf16)
                    if pend is None:
                        for m in range(NQK):
                            emit_qkv_m(xt_sb, qk_sb, m)
                        for g in range(6):
                            emit_v_g(xt_sb, v_sb, g)
                        emit_v_ones(v_sb)
                    else:
                        attn_window(xt_sb, qk_sb, v_sb)
                    pend = (qk_sb, v_sb, b)
                # drain: attention of the last batch, then the two
                # outstanding projections.
                attn_window(None, None, None)
                for m in range(NCT):
                    emit_pj(*projq[0], m=m)

            if reps == 1:
                body()
            elif reps == 0:
                with tc.For_i(0, rv, 1) as _i:
                    body(_i)
            else:
                with tc.For_i(0, reps, 1) as _i:
                    body(_i)

    nc.compile()
    return nc


_NC_CACHE = {}


def _get_nc(bpc: int = BPC):
    if bpc not in _NC_CACHE:
        _NC_CACHE[bpc] = build_bass(bpc)
    return _NC_CACHE[bpc]


def make_in_maps(x1, x2, qkv_w, proj_w, proj_b, n_cores=N_CORES):
    x1 = np.asarray(x1, dtype=np.float32)
    x2 = np.asarray(x2, dtype=np.float32)
    qkv_w = np.asarray(qkv_w, dtype=np.float32)
    proj_w = np.asarray(proj_w, dtype=np.float32)
    proj_b = np.asarray(proj_b, dtype=np.float32)

    b = x1.shape[0]
    xt = np.empty((b, C, N), dtype=NP_BF16)
    xt[:, :, :T] = x1[:, :T, :].transpose(0, 2, 1).astype(NP_BF16)
    xt[:, :, T:] = x2[:, T:, :].transpose(0, 2, 1).astype(NP_BF16)

    wqk = np.ascontiguousarray(qkv_w[:2 * C].T).astype(NP_BF16)
    wv = np.ascontiguousarray(qkv_w[2 * C:].T).astype(NP_BF16)
    wp = np.ascontiguousarray(proj_w.T).astype(NP_BF16)
    pbt = np.ascontiguousarray(proj_b.reshape(NCT, 128).T)  # [128, NCT] f32

    bpc = b // n_cores
    return [
        {"xt": xt[c * bpc:(c + 1) * bpc], "wqk": wqk, "wv": wv, "wp": wp,
         "pbt": pbt}
        for c in range(n_cores)
    ], bpc


def kernel(x1, x2, qkv_w, proj_w, proj_b):
    in_maps, bpc = make_in_maps(x1, x2, qkv_w, proj_w, proj_b)
    nc = _get_nc(bpc)
    res = run_bass_kernel_spmd(nc, in_maps, list(range(N_CORES)))
    yt = np.concatenate([res.results[c]["y"] for c in range(N_CORES)], axis=0)
    return np.ascontiguousarray(yt.transpose(0, 2, 1))
